# revision 30
# baseline (speedup 1.0000x reference)
"""Tied-attention (MSA-style) kernel for 8 TRN2 NeuronCores.

Problem: x (32,1024,256) f32; q/kv projections; tied attention over the
r=32 MSA-row dim (logits summed over r); softmax; out-projection + bias.

Sharding: tensor-parallel by heads (8 heads -> 1 head per core). Each core
computes q/k/v for its head from the full (host-pre-transposed, bf16-cast)
x, accumulates its head's tied logits S^T = sum_r k_r q_r^T entirely
locally (no collective), softmaxes along the PSUM partition axis via a
ones-matmul, applies attention, then two AllToAlls redistribute per-head
outputs into per-core row shards (4 rows of r each); the first one
overlaps the second half of the attention compute. Each core finishes
with the full output projection for its rows; the host concatenates and
un-transposes the shards.

v is produced head-transposed and flipped to row-major with PE-mode
transposes (DMA transpose serializes the whole DMA subsystem via
xbar_mode transitions - measured 12us/pair stalls - so it is avoided).

Compute dtype: bf16 operands, f32 PSUM accumulation, f32 softmax.
"""
import numpy as np
import ml_dtypes

import concourse.bacc as bacc
import concourse.mybir as mybir
import concourse.tile as tile
from concourse.bass_utils import run_bass_kernel_spmd

dt = mybir.dt
BF16 = ml_dtypes.bfloat16

H, D, R, N, DIM = 8, 64, 32, 1024, 256
INNER = H * D          # 512
ROWS = R * N           # 32768
NPAIR = R // 2         # 16
NCORES = 8
RL = R // NCORES       # 4 rows of r per core after AllToAll
SCALE = (D ** -0.5) * (R ** -0.5)

_NC_CACHE = None


def _build():
    nc = bacc.Bacc("TRN2", target_bir_lowering=False, debug=False, num_devices=NCORES)

    xt = nc.dram_tensor("xt", [DIM, ROWS], dt.bfloat16, kind="ExternalInput")
    wq = nc.dram_tensor("wq", [DIM, D], dt.bfloat16, kind="ExternalInput")
    wk = nc.dram_tensor("wk", [DIM, D], dt.bfloat16, kind="ExternalInput")
    wv = nc.dram_tensor("wv", [DIM, D], dt.bfloat16, kind="ExternalInput")
    wout = nc.dram_tensor("wout", [INNER, DIM], dt.bfloat16, kind="ExternalInput")
    bias = nc.dram_tensor("bias", [128, 2], dt.float32, kind="ExternalInput")
    ident = nc.dram_tensor("ident", [128, 128], dt.bfloat16, kind="ExternalInput")
    yt = nc.dram_tensor("yt", [DIM, RL * N], dt.float32, kind="ExternalOutput")

    with tile.TileContext(nc) as tc:
        with (
            tc.tile_pool(name="dram", bufs=1, space="DRAM") as dram,
            tc.tile_pool(name="persist", bufs=1) as per,
            tc.tile_pool(name="xc", bufs=4) as xcp,
            tc.tile_pool(name="stage", bufs=4) as stg,
            tc.tile_pool(name="gio", bufs=3) as gio,
        ):
            # A2A chunks over pairs 0-7 / 8-11 / 12-15 (rows [0,16) /
            # [16,24) / [24,32)); dest d gets rows {d, d+8} / {d+16} /
            # {d+24}. Chunk 0 hides under the second half of the attention
            # compute; the 1MB chunks 1-2 keep the exposed tail short.
            a2a_ins = [dram.tile([NCORES, 2, D, N], dt.bfloat16, name="a2ai0"),
                       dram.tile([NCORES, 1, D, N], dt.bfloat16, name="a2ai1"),
                       dram.tile([NCORES, 1, D, N], dt.bfloat16, name="a2ai2")]
            a2a_outs = [dram.tile([NCORES, 2, D, N], dt.bfloat16, name="a2ao0"),
                        dram.tile([NCORES, 1, D, N], dt.bfloat16, name="a2ao1"),
                        dram.tile([NCORES, 1, D, N], dt.bfloat16, name="a2ao2")]

            # persistent SBUF tensors
            wq_sb = per.tile([128, 2, D], dt.bfloat16, tag="wq")
            wk_sb = per.tile([128, 2, D], dt.bfloat16, tag="wk")
            wv_sb = per.tile([128, 2, D], dt.bfloat16, tag="wv")
            wout_sb = per.tile([128, 4, DIM], dt.bfloat16, tag="wout")
            bias_sb = per.tile([128, 2], dt.float32, tag="bias")
            ident_sb = per.tile([128, 128], dt.bfloat16, tag="ident")
            ones_col = per.tile([128, 1], dt.bfloat16, tag="ones_col")
            den_sb = per.tile([1, N], dt.float32, tag="den")
            bcf_sb = per.tile([128, N], dt.float32, tag="bcf")
            # per-pair persistent: q^T/k^T [(r-parity, d), n], v row-major
            qts = [per.tile([128, N], dt.bfloat16, tag=f"qt{p}", name=f"qt{p}")
                   for p in range(NPAIR)]
            kts = [per.tile([128, N], dt.bfloat16, tag=f"kt{p}", name=f"kt{p}")
                   for p in range(NPAIR)]
            vs = [per.tile([128, 8, 128], dt.bfloat16, tag=f"v{p}", name=f"v{p}")
                  for p in range(NPAIR)]
            # P^T tiles per jc: [j-in-chunk, i]
            pts = [per.tile([128, N], dt.bfloat16, tag=f"pt{jc}", name=f"pt{jc}")
                   for jc in range(8)]

            nc.gpsimd.dma_start(wq_sb[:], wq.ap().rearrange("(a p) m -> p a m", p=128))
            nc.gpsimd.dma_start(wk_sb[:], wk.ap().rearrange("(a p) m -> p a m", p=128))
            nc.gpsimd.dma_start(wv_sb[:], wv.ap().rearrange("(a p) m -> p a m", p=128))
            nc.gpsimd.dma_start(wout_sb[:], wout.ap().rearrange("(a p) m -> p a m", p=128))
            nc.gpsimd.dma_start(bias_sb[:], bias[:])
            nc.gpsimd.dma_start(ident_sb[:], ident[:])
            nc.vector.memset(ones_col[:], 1.0)

            # warm-up collective: absorbs cross-core start skew and ncfw cold
            # init while phase 1 computes; CC engine only.
            warm_in = dram.tile([1, 64], dt.float32, name="warm_in")
            warm_out = dram.tile([NCORES, 64], dt.float32, name="warm_out")
            nc.gpsimd.collective_compute(
                "AllGather",
                mybir.AluOpType.bypass,
                replica_groups=[list(range(NCORES))],
                ins=[warm_in.opt()],
                outs=[warm_out.opt()],
            )

            # ---- Phase 1: projections q^T,k^T (parity layout) + v (row major) ----
            with (
                tc.tile_pool(name="ps_proj", bufs=3, space="PSUM") as psp,
                tc.tile_pool(name="ps_vtr", bufs=2, space="PSUM") as psv,
            ):
                for p in range(NPAIR):
                    xc = [xcp.tile([128, 2 * N], dt.bfloat16, tag="xc", name=f"xc{p}_{i}")
                          for i in range(2)]
                    for kt in range(2):
                        for hf in range(2):
                            nc.sync.dma_start(
                                xc[kt][:, hf * N:(hf + 1) * N],
                                xt[kt * 128:(kt + 1) * 128,
                                   (2 * p + hf) * N:(2 * p + hf + 1) * N])
                    pq = psp.tile([128, N], dt.float32, tag="proj", name=f"pq{p}")
                    pk = psp.tile([128, N], dt.float32, tag="proj", name=f"pk{p}")
                    pv = psp.tile([128, N], dt.float32, tag="proj", name=f"pv{p}")
                    for w_sb, ps in ((wq_sb, pq), (wk_sb, pk), (wv_sb, pv)):
                        for kt in range(2):
                            for col, base in ((0, 0), (64, N)):
                                for nh in range(2):
                                    sl = slice(base + nh * 512, base + nh * 512 + 512)
                                    nc.tensor.matmul(
                                        ps[col:col + 64, nh * 512:nh * 512 + 512],
                                        w_sb[:, kt, :], xc[kt][:, sl],
                                        start=(kt == 0), stop=(kt == 1),
                                        tile_position=(0, col))
                    nc.scalar.activation(qts[p][:], pq[:],
                                         mybir.ActivationFunctionType.Copy)
                    nc.vector.tensor_copy(kts[p][:], pk[:])
                    vstage = stg.tile([128, N], dt.bfloat16, tag="vstage",
                                      name=f"vst{p}")
                    nc.vector.tensor_copy(vstage[:], pv[:])
                    # PE-transpose v^T (parity,d)xn -> n x (parity,d)
                    for jc in range(8):
                        pt_ps = psv.tile([128, 128], dt.bfloat16, tag="vtr",
                                         name=f"vtr{p}_{jc}")
                        nc.tensor.transpose(pt_ps[:],
                                            vstage[:, jc * 128:(jc + 1) * 128],
                                            ident_sb[:])
                        if jc % 2 == 0:
                            nc.vector.tensor_copy(vs[p][:, jc, :], pt_ps[:])
                        else:
                            nc.scalar.activation(vs[p][:, jc, :], pt_ps[:],
                                                 mybir.ActivationFunctionType.Copy)

            # ---- Phase 2: S^T = sum_r k_r q_r^T (per j-chunk), softmax ----
            with (
                tc.tile_pool(name="ps_s", bufs=3, space="PSUM") as pss,
                tc.tile_pool(name="ps_den", bufs=1, space="PSUM") as psd,
            ):
                pden = psd.tile([1, N], dt.float32, tag="den")
                for jc in range(8):
                    ps = pss.tile([128, N], dt.float32, tag="s", name=f"s{jc}")
                    for p in range(NPAIR):
                        for ih in range(2):
                            nc.tensor.matmul(
                                ps[:, ih * 512:ih * 512 + 512],
                                kts[p][:, jc * 128:(jc + 1) * 128],
                                qts[p][:, ih * 512:ih * 512 + 512],
                                start=(p == 0), stop=(p == NPAIR - 1))
                    nc.scalar.activation(pts[jc][:], ps[:],
                                         mybir.ActivationFunctionType.Exp,
                                         scale=SCALE)
                    for ih in range(2):
                        nc.tensor.matmul(pden[:, ih * 512:ih * 512 + 512],
                                         ones_col[:],
                                         pts[jc][:, ih * 512:ih * 512 + 512],
                                         start=(jc == 0), stop=(jc == 7))
                nc.scalar.activation(den_sb[:], pden[:],
                                     mybir.ActivationFunctionType.Copy)
            # broadcast first, then full-width reciprocal (fast on 128 lanes)
            nc.gpsimd.partition_broadcast(bcf_sb[:], den_sb[:])
            nc.vector.reciprocal(bcf_sb[:], bcf_sb[:])

            # ---- Phase 3 + 4: attention-weighted values; two overlapped A2As.
            # Even pairs p feed A2A chunk 0 (rows 4d,4d+1), odd pairs chunk 1
            # (rows 4d+2,4d+3); chunk 0's collective overlaps odd-pair compute.
            with tc.tile_pool(name="ps_av", bufs=4, space="PSUM") as psa:
                for half, prange in ((0, range(0, 8)), (1, range(8, 12)),
                                     (2, range(12, 16))):
                    for p in prange:
                        po = psa.tile([128, N], dt.float32, tag="av", name=f"av{p}")
                        for jc in range(8):
                            for ih in range(2):
                                nc.tensor.matmul(
                                    po[:, ih * 512:ih * 512 + 512],
                                    vs[p][:, jc, :],
                                    pts[jc][:, ih * 512:ih * 512 + 512],
                                    start=(jc == 0), stop=(jc == 7))
                        osb = stg.tile([128, N], dt.bfloat16, tag="osb",
                                       name=f"osb{p}")
                        # normalize by the softmax denominator on evacuation
                        nc.vector.tensor_mul(osb[:], po[:], bcf_sb[:])
                        for half_row in range(2):
                            r = 2 * p + half_row
                            rg = r // 8
                            nc.sync.dma_start(
                                a2a_ins[half][r % 8, rg if rg < 2 else 0, :, :],
                                osb[64 * half_row:64 * half_row + 64, :])
                    nc.gpsimd.collective_compute(
                        "AllToAll",
                        mybir.AluOpType.bypass,
                        replica_groups=[list(range(NCORES))],
                        ins=[a2a_ins[half].opt()],
                        outs=[a2a_outs[half].opt()],
                    )

            # ---- Phase 5: y^T = Wout^T out + bias for own 4 r-rows ----
            with tc.tile_pool(name="ps_y", bufs=4, space="PSUM") as psy:
                for rl in range(RL):
                    half, sub = (0, rl) if rl < 2 else (rl - 1, 0)
                    g = gio.tile([128, 4, N], dt.bfloat16, tag="g", name=f"g{rl}")
                    for kt in range(4):
                        nc.scalar.dma_start(g[0:64, kt, :],
                                            a2a_outs[half][2 * kt, sub, :, :])
                        nc.scalar.dma_start(g[64:128, kt, :],
                                            a2a_outs[half][2 * kt + 1, sub, :, :])
                    for m in range(2):
                        sl_m = slice(m * 128, m * 128 + 128)
                        py = psy.tile([128, N], dt.float32, tag="y",
                                      name=f"py{rl}_{m}")
                        for kt in range(4):
                            for nh in range(2):
                                nc.tensor.matmul(py[:, nh * 512:nh * 512 + 512],
                                                 wout_sb[:, kt, sl_m],
                                                 g[:, kt, nh * 512:nh * 512 + 512],
                                                 start=(kt == 0), stop=(kt == 3))
                        ysb = gio.tile([128, N], dt.float32, tag="ysb",
                                       name=f"ysb{rl}_{m}")
                        if m == 0:
                            nc.vector.tensor_scalar_add(ysb[:], py[:],
                                                        bias_sb[:, m:m + 1])
                        else:
                            nc.scalar.activation(ysb[:], py[:],
                                                 mybir.ActivationFunctionType.Identity,
                                                 bias=bias_sb[:, m:m + 1])
                        nc.gpsimd.dma_start(yt[sl_m, rl * N:(rl + 1) * N], ysb[:])
    nc.finalize()
    return nc


def kernel(x, Wq, Wkv, Wout, bout, tie_attn_dim):
    global _NC_CACHE
    assert int(tie_attn_dim) == R
    x = np.asarray(x, dtype=np.float32)
    xt = np.ascontiguousarray(x.reshape(ROWS, DIM).T).astype(BF16)
    wout_b = np.asarray(Wout, np.float32).astype(BF16)
    bias_b = np.ascontiguousarray(np.asarray(bout, np.float32).reshape(2, 128).T)
    ident = np.eye(128, dtype=BF16)
    Wq = np.asarray(Wq, np.float32)
    Wkv = np.asarray(Wkv, np.float32)

    in_maps = []
    for c in range(NCORES):
        sl = slice(c * D, (c + 1) * D)
        in_maps.append({
            "xt": xt,
            "wq": np.ascontiguousarray(Wq[:, sl]).astype(BF16),
            "wk": np.ascontiguousarray(Wkv[:, sl]).astype(BF16),
            "wv": np.ascontiguousarray(Wkv[:, INNER + c * D:INNER + (c + 1) * D]).astype(BF16),
            "wout": wout_b,
            "bias": bias_b,
            "ident": ident,
        })

    if _NC_CACHE is None:
        _NC_CACHE = _build()
    last_err = None
    for _attempt in range(3):
        try:
            res = run_bass_kernel_spmd(_NC_CACHE, in_maps,
                                       core_ids=list(range(NCORES)))
            break
        except Exception as e:  # transient NRT device errors; retry
            last_err = e
    else:
        raise last_err

    y = np.empty((R, N, DIM), dtype=np.float32)
    for c in range(NCORES):
        ytc = res.results[c]["yt"].reshape(DIM, RL, N)  # row-group rg = r//8
        for rg in range(RL):
            y[c + 8 * rg] = ytc[:, rg, :].T
    return y
